# revision 1
# baseline (speedup 1.0000x reference)
"""Trainium2 Bass kernel: GroupNorm + single-head self-attention block.

Reference computation (per batch b):
    xn = GroupNorm(x, 16 groups, eps=1e-5) * gamma + beta
    q/k/v = W @ xn + b          (1x1 conv == channel matmul), [C, N]
    S = (q^T k) / sqrt(C)       [N, N]
    A = softmax_j(S)
    O = v @ A^T                 [C, N]
    y = wo @ O + bo + x

Shapes: B=4, C=256, H=W=64 -> N=4096.

Sharding: 8 cores = 4 batches x 2 query-halves.  Each core receives the
full x[b] with its query half permuted to the front, computes xn / v
for all N keys (cheap, avoids any collectives) and runs attention for
its 2048 queries.  The device program is identical on all cores (SPMD).

Algebraic restructuring (host-side, exact):
  - S^T[j,i] = sum_c k[c,j] q[c,i] = xn^T WQK xn with WQK = wq^T wk
    folded on the host; the per-query bias term from bk shifts all
    scores of a query equally and is dropped (softmax-invariant), the
    bq term survives as bqk = wk^T bq.
  - wo is folded into v: out = wo (v A_n^T) = (WOV xn + wo bv) A_n^T
    with WOV = wo wv.  The attention-value matmul then directly
    produces the final projection.

Device algorithm (per core):
  - GroupNorm stats via bn_stats/bn_aggr per channel + PE matmul with a
    group-indicator matrix for the cross-partition (channel) reduction.
  - qk = WQK^T xn + bqk for the 2048 local queries.
  - Scores computed TRANSPOSED per key-tile: S^T = xn^T qk, so both
    operands are natural [C, *] layouts (no transposes anywhere).
  - exp without max-subtraction (scores ~ N(0,1); fp32 exp is safe).
  - softmax denominator: ones-vector matmul over partitions on PE,
    broadcast back via a 0-stride-partition DMA, reciprocal on DVE.
  - out = v'^T A^T accumulated in PSUM, then *recip + residual on DVE.

Big matmuls run in float32r (full-rate fp32 PE mode).  fp32r operands
must be produced "rounded" by a compute engine, so every matmul input
tile is written by DVE/ACT with a float32r output dtype.
"""

import sys

sys.path.insert(0, "/opt/trn_rl_repo")

from contextlib import ExitStack

import numpy as np

import concourse.bacc as bacc
import concourse.bass as bass
import concourse.mybir as mybir
import concourse.tile as tile

B, C, H, W = 4, 256, 64, 64
N = H * W              # keys per batch
GROUPS = 16
EPS = 1e-5
NCORES = 8
QSPLIT = NCORES // B   # query shards per batch
NQ = N // QSPLIT       # queries per core
P = 128
CCH = C // P           # channel chunks (2)
IB = 512               # query block (one PSUM bank of f32)
NIB = NQ // IB         # query blocks per core
NJT = N // P           # key tiles (32)
GSZ = C // GROUPS      # channels per group (16)

F32 = mybir.dt.float32
F32R = mybir.dt.float32r
AF = mybir.ActivationFunctionType
OP = mybir.AluOpType


def build_nc(mm_dtype: str = "f32r"):
    """Emit the single-core SPMD program."""
    fp8_dr = mm_dtype.endswith("+fp8")
    base = mm_dtype.replace("+fp8", "")
    DTM = {"f32r": F32R, "bf16": mybir.dt.bfloat16, "f32": F32}[base]
    FP8 = mybir.dt.float8e4
    DTV = FP8 if fp8_dr else DTM   # dtype of the at / v' operands
    nc = bacc.Bacc()

    x_d = nc.declare_dram_parameter("x", [C, N], F32, isOutput=False)
    wqk_d = nc.declare_dram_parameter("wqk", [C, C], F32, isOutput=False)
    wovT_d = nc.declare_dram_parameter("wovT", [C, C], F32, isOutput=False)
    gamma_d = nc.declare_dram_parameter("gamma", [C], F32, isOutput=False)
    beta_d = nc.declare_dram_parameter("beta", [C], F32, isOutput=False)
    bqk_d = nc.declare_dram_parameter("bqk", [C], F32, isOutput=False)
    bvp_d = nc.declare_dram_parameter("bvp", [C], F32, isOutput=False)
    bo_d = nc.declare_dram_parameter("bo", [C], F32, isOutput=False)
    gind_d = nc.declare_dram_parameter("gind", [CCH, P, GROUPS], F32, isOutput=False)
    gindT_d = nc.declare_dram_parameter("gindT", [CCH, GROUPS, P], F32, isOutput=False)
    y_d = nc.declare_dram_parameter("y", [C, NQ], F32, isOutput=True)

    with tile.TileContext(nc) as tc, ExitStack() as ctx:
        const = ctx.enter_context(tc.tile_pool(name="const", bufs=1))
        data = ctx.enter_context(tc.tile_pool(name="data", bufs=1))

        # ---- weights: DMA to f32 staging, DVE-copy to fp32r tiles ----
        stage = ctx.enter_context(tc.tile_pool(name="stage", bufs=1))

        # fp32r lhsT free-dim counts must be even -> ones "column" is [P, 2]
        # (memset cannot emit fp32r; stage in f32 and DVE-copy to round)
        ones_f = const.tile([P, P], F32, name="ones_f")
        nc.vector.memset(ones_f, 1.0)
        ones_col2 = const.tile([P, 2], DTM, name="ones_col2")
        nc.vector.tensor_copy(ones_col2, ones_f[:, 0:2])
        ones_row_r = const.tile([1, P], DTM, name="ones_row_r")
        nc.vector.tensor_copy(ones_row_r, ones_f[0:1, :])
        if fp8_dr:
            # DoubleRow ones "column": [K, 2 pair-slices, M=16] -- the pair
            # dim stride must be 16B-aligned, so M is padded to 16
            ones_dr = const.tile([P, 2, 16], FP8, name="ones_dr")
            nc.vector.tensor_copy(
                ones_dr, ones_f[:, 0:32].rearrange("p (a b) -> p a b", a=2)
            )
            neg_ln16 = const.tile([P, 1], F32, name="neg_ln16")
            nc.vector.memset(neg_ln16, -2.772588722239781)  # -ln(16)
        # PE HAM warm-up scaffolding: the clock gate only reaches 2.4 GHz
        # after ~3.4us of sustained activity and re-throttles after an idle
        # window, so burn dummy matmuls during the DMA/GroupNorm prologue
        # (PE is otherwise idle there) and drip data-dependent "pings" so
        # the gate never sees an idle window before the real matmuls start.
        warm_src_f = const.tile([P, 512], F32, name="warm_src_f")
        nc.vector.memset(warm_src_f, 0.0)
        warm_src = const.tile([P, 512], DTM, name="warm_src")
        nc.vector.tensor_copy(warm_src, warm_src_f)
        def load_w(handle, nm):
            tiles = []
            for ch in range(CCH):
                s = stage.tile([P, C], F32, name=f"{nm}{ch}_s", tag=f"{nm}{ch}_s")
                nc.scalar.dma_start(out=s, in_=handle[ch * P:(ch + 1) * P, :])
                t = const.tile([P, C], DTM, name=f"{nm}{ch}")
                nc.vector.tensor_copy(t, s)
                tiles.append(t)
            return tiles

        wqk = load_w(wqk_d, "wqk")      # [c, c'] chunks; lhsT for qk proj
        wovT = load_w(wovT_d, "wovT")   # [c', o] chunks; rhs for v' proj

        def load_vec(handle, nm):
            tiles = []
            for ch in range(CCH):
                t = const.tile([P, 1], F32, name=f"{nm}{ch}")
                nc.scalar.dma_start(
                    out=t, in_=handle[ch * P:(ch + 1) * P].unsqueeze(1)
                )
                tiles.append(t)
            return tiles

        gamma = load_vec(gamma_d, "gamma")
        beta = load_vec(beta_d, "beta")
        bqk = load_vec(bqk_d, "bqk")
        bo = load_vec(bo_d, "bo")

        bvp_s = stage.tile([1, C], F32, name="bvp_s")
        nc.scalar.dma_start(out=bvp_s, in_=bvp_d[:].unsqueeze(0))
        bvp_row = const.tile([1, C], DTM, name="bvp_row")
        nc.vector.tensor_copy(bvp_row, bvp_s)

        gind = []
        gindT = []
        for ch in range(CCH):
            gi = const.tile([P, GROUPS], F32, name=f"gind{ch}")
            nc.scalar.dma_start(out=gi, in_=gind_d[ch])
            gind.append(gi)
            gt = const.tile([GROUPS, P], F32, name=f"gindT{ch}")
            nc.scalar.dma_start(out=gt, in_=gindT_d[ch])
            gindT.append(gt)


        # ---- x in (staging pool released after GroupNorm) ----
        xn = data.tile([P, CCH, N], DTM, name="xn")
        resid = data.tile([P, CCH, NQ], F32, name="resid")


        with tc.tile_pool(name="xf_pool", bufs=1) as xf_pool, \
             tc.tile_pool(name="gn_psum", bufs=1, space="PSUM") as gn_psum, \
             tc.tile_pool(name="warm_psum", bufs=1, space="PSUM") as warm_psum, \
             tc.tile_pool(name="gn_sb", bufs=1) as gn_sb:
            warm_ps = warm_psum.tile([P, 512], F32, name="warm_ps")

            def warm(rhs=None, n=1):
                # M=2 keeps the HAM activity monitor fed at ~1/64th of the
                # PE-array power (wide bursts trip the firmware throttle)
                for _ in range(n):
                    nc.tensor.matmul(
                        warm_ps[:2, :512] if rhs is None else warm_ps[:2, :rhs.shape[-1]],
                        lhsT=ones_col2 if rhs is None else ones_f[:, 0:2],
                        rhs=warm_src if rhs is None else rhs,
                        start=True, stop=True, skip_group_check=True,
                    )

            warm(n=26)  # ~3.5us+ dense burst at t=0 -> gate opens early
            xf = xf_pool.tile([P, CCH, N], F32, name="xf")
            NS = N // 512  # bn_stats subgroups; DMA per subgroup to overlap
            for ch in range(CCH):
                for sg in range(NS):
                    eng = nc.sync if (ch * NS + sg) % 2 == 0 else nc.gpsimd
                    eng.dma_start(
                        out=xf[:, ch, sg * 512:(sg + 1) * 512],
                        in_=x_d[ch * P:(ch + 1) * P, sg * 512:(sg + 1) * 512],
                    )
            # ---- GroupNorm stats ----
            pc = []  # per-channel [mean, mean^2 + var] per chunk
            for ch in range(CCH):
                st6 = gn_sb.tile([P, NS, 6], F32, name=f"st6_{ch}")
                for sg in range(NS):
                    nc.vector.bn_stats(
                        out=st6[:, sg, :], in_=xf[:, ch, sg * 512:(sg + 1) * 512]
                    )
                    warm(rhs=st6[:, sg, :])
                mv = gn_sb.tile([P, 2], F32, name=f"mv{ch}")
                nc.vector.bn_aggr(out=mv, in_=st6)
                pcs = gn_sb.tile([P, 2], F32, name=f"pcs{ch}")
                nc.vector.tensor_copy(pcs[:, 0:1], mv[:, 0:1])
                # pcs[:,1] = mean^2 + var  (-> group E[x^2] after averaging)
                msq = gn_sb.tile([P, 1], F32, name=f"msq{ch}")
                nc.vector.tensor_mul(msq, mv[:, 0:1], mv[:, 0:1])
                nc.vector.tensor_add(pcs[:, 1:2], mv[:, 1:2], msq)
                pc.append(pcs)

            # residual (+ bo) for the local query half (ACT is idle here)
            for ch in range(CCH):
                nc.scalar.activation(
                    out=resid[:, ch, :], in_=xf[:, ch, :NQ], func=AF.Identity,
                    bias=bo[ch], scale=1.0,
                )

            gs_ps = gn_psum.tile([GROUPS, 2], F32, name="gs_ps")
            for ch in range(CCH):
                nc.tensor.matmul(
                    gs_ps, lhsT=gind[ch], rhs=pc[ch],
                    start=(ch == 0), stop=(ch == CCH - 1),
                )
            # per-channel stats are already means -> average over the GSZ
            # channels of each group
            gs = gn_sb.tile([GROUPS, 2], F32, name="gs")
            nc.scalar.mul(gs, gs_ps, 1.0 / GSZ)
            gvar = gn_sb.tile([GROUPS, 1], F32, name="gvar")
            gmsq = gn_sb.tile([GROUPS, 1], F32, name="gmsq")
            nc.vector.tensor_mul(gmsq, gs[:, 0:1], gs[:, 0:1])
            nc.vector.tensor_sub(gvar, gs[:, 1:2], gmsq)
            # rstd = 1/sqrt(var+eps)
            gstd = gn_sb.tile([GROUPS, 1], F32, name="gstd")
            eps_t = gn_sb.tile([GROUPS, 1], F32, name="eps_t")
            nc.vector.memset(eps_t, EPS)
            nc.scalar.activation(
                out=gstd, in_=gvar, func=AF.Sqrt, bias=eps_t, scale=1.0
            )
            gmr = gn_sb.tile([GROUPS, 2], F32, name="gmr")
            nc.vector.tensor_copy(gmr[:, 0:1], gs[:, 0:1])
            nc.vector.reciprocal(gmr[:, 1:2], gstd)

            # broadcast group (mean, rstd) back to channels, build affine
            for ch in range(CCH):
                cb_ps = gn_psum.tile([P, 2], F32, name="cb_ps", tag="cb_ps")
                nc.tensor.matmul(cb_ps, lhsT=gindT[ch], rhs=gmr,
                                 start=True, stop=True)
                cb = gn_sb.tile([P, 2], F32, name=f"cb{ch}")
                nc.vector.tensor_copy(cb, cb_ps)
                scale = gn_sb.tile([P, 1], F32, name=f"scale{ch}")
                nc.vector.tensor_mul(scale, gamma[ch], cb[:, 1:2])
                shift = gn_sb.tile([P, 1], F32, name=f"shift{ch}")
                nc.vector.tensor_mul(shift, cb[:, 0:1], scale)
                nc.vector.tensor_sub(shift, beta[ch], shift)
                # xn = x * scale + shift (column blocks -> projections
                # on early columns can start while later ones convert)
                for xb in range(4):
                    xsl = slice(xb * (N // 4), (xb + 1) * (N // 4))
                    nc.vector.tensor_scalar(
                        out=xn[:, ch, xsl], in0=xf[:, ch, xsl],
                        scalar1=scale, scalar2=shift, op0=OP.mult, op1=OP.add,
                    )
                warm(rhs=cb)

        # ---- projections ----
        qk = data.tile([P, CCH, NQ], DTM, name="qk")    # WQK^T xn + bqk
        vT = data.tile([P, NJT, C], DTV, name="vT")     # (WOV xn)^T + wo bv

        with tc.tile_pool(name="pj_psum", bufs=3, space="PSUM") as pj_psum:
            # v'-bias row broadcast once: b_sb[j, o] = bvp[o]
            bps = pj_psum.tile([P, C], F32, name="bps", tag="vT_ps")
            nc.tensor.matmul(bps, lhsT=ones_row_r, rhs=bvp_row,
                             start=True, stop=True)
            b_sb = const.tile([P, C], F32, name="b_sb")
            nc.vector.tensor_copy(b_sb, bps)
            # qk[c', i] = sum_c WQK[c, c'] xn[c, i] + bqk[c']
            for oc in range(CCH):
                for it in range(NQ // 512):
                    ps = pj_psum.tile([P, 512], F32, name="qk_ps", tag="qk_ps")
                    for ch in range(CCH):
                        nc.tensor.matmul(
                            ps,
                            lhsT=wqk[ch][:, oc * P:(oc + 1) * P],
                            rhs=xn[:, ch, it * 512:(it + 1) * 512],
                            start=(ch == 0), stop=(ch == CCH - 1),
                        )
                    nc.vector.tensor_scalar_add(
                        qk[:, oc, it * 512:(it + 1) * 512], ps, scalar1=bqk[oc]
                    )
            # vT[j, o] = sum_c' xn[c', j] WOV[o, c'] + (wo bv)[o]
            for jt in range(NJT):
                ps = pj_psum.tile([P, C], F32, name="vT_ps", tag="vT_ps")
                for ch in range(CCH):
                    nc.tensor.matmul(
                        ps,
                        lhsT=xn[:, ch, jt * P:(jt + 1) * P],
                        rhs=wovT[ch],
                        start=(ch == 0), stop=(ch == CCH - 1),
                    )
                nc.vector.tensor_add(vT[:, jt, :], ps, b_sb)

        # ---- attention ----
        with tc.tile_pool(name="st_psum", bufs=2, space="PSUM") as st_psum, \
             tc.tile_pool(name="o_psum", bufs=1, space="PSUM") as o_psum, \
             tc.tile_pool(name="sm_psum", bufs=1, space="PSUM") as sm_psum, \
             tc.tile_pool(name="at_pool", bufs=6) as at_pool, \
             tc.tile_pool(name="fin", bufs=2) as fin:
            for ib in range(NIB):
                isl = slice(ib * IB, (ib + 1) * IB)
                sums_ps = sm_psum.tile(
                    [16 if fp8_dr else 2, IB], F32, name="sums_ps", tag="sums"
                )
                o_ps = [
                    o_psum.tile([P, IB], F32, name=f"o_ps{cc}", tag=f"o{cc}")
                    for cc in range(CCH)
                ]
                # Software-pipelined on key-tile PAIRS: the score PSUM
                # tile holds two key-tiles (2 banks) so ONE exp covers the
                # pair and writes the fp8 DoubleRow [K, 2, N] layout
                # directly.  DR matmuls consume the pair with a 1-pair lag
                # so their waits are pre-satisfied.
                if fp8_dr:
                    PLAG = 1
                    npair = NJT // 2
                    ats = {}
                    for p in range(npair + PLAG):
                        if p < npair:
                            stp = st_psum.tile([P, 2, IB], F32, name="stp", tag="st")
                            for m in range(2):
                                jt = 2 * p + m
                                jsl = slice(jt * P, (jt + 1) * P)
                                for ch in range(CCH):
                                    nc.tensor.matmul(
                                        stp[:, m, :],
                                        lhsT=xn[:, ch, jsl],
                                        rhs=qk[:, ch, isl],
                                        start=(ch == 0), stop=(ch == CCH - 1),
                                    )
                            atp = at_pool.tile([P, 2, IB], FP8, name="atp", tag="at")
                            # A^T = exp(S^T/16 - ln 16); the -ln16 keeps fp8e4
                            # in range and cancels in the normalization
                            nc.scalar.activation(
                                out=atp.rearrange("p a b -> p (a b)"),
                                in_=stp.rearrange("p a b -> p (a b)"),
                                func=AF.Exp, scale=1.0 / 16.0, bias=neg_ln16,
                            )
                            ats[p] = atp
                        if p >= PLAG:
                            pg = p - PLAG
                            atp = ats.pop(pg)
                            nc.tensor.matmul(
                                sums_ps, lhsT=ones_dr, rhs=atp,
                                start=(pg == 0), stop=(pg == npair - 1),
                                perf_mode=mybir.MatmulPerfMode.DoubleRow,
                            )
                            for cc in range(CCH):
                                nc.tensor.matmul(
                                    o_ps[cc],
                                    lhsT=vT[:, 2 * pg:2 * pg + 2,
                                            cc * P:(cc + 1) * P],
                                    rhs=atp,
                                    start=(pg == 0), stop=(pg == npair - 1),
                                    perf_mode=mybir.MatmulPerfMode.DoubleRow,
                                )
                else:
                    LAG = 2
                    ats = {}
                    for jt in range(NJT + LAG):
                        if jt < NJT:
                            jsl = slice(jt * P, (jt + 1) * P)
                            st = st_psum.tile([P, IB], F32, name="st", tag="st")
                            for ch in range(CCH):
                                nc.tensor.matmul(
                                    st,
                                    lhsT=xn[:, ch, jsl],
                                    rhs=qk[:, ch, isl],
                                    start=(ch == 0), stop=(ch == CCH - 1),
                                )
                            at = at_pool.tile([P, IB], DTM, name="at", tag="at")
                            nc.scalar.activation(
                                out=at, in_=st, func=AF.Exp, scale=1.0 / 16.0
                            )
                            ats[jt] = at
                        if jt >= LAG and (jt - LAG) % 2 == 1:
                            for g in (jt - LAG - 1, jt - LAG):
                                at_g = ats.pop(g)
                                nc.tensor.matmul(
                                    sums_ps, lhsT=ones_col2, rhs=at_g,
                                    start=(g == 0), stop=(g == NJT - 1),
                                )
                                for cc in range(CCH):
                                    nc.tensor.matmul(
                                        o_ps[cc],
                                        lhsT=vT[:, g, cc * P:(cc + 1) * P],
                                        rhs=at_g,
                                        start=(g == 0), stop=(g == NJT - 1),
                                    )

                # free the accumulators quickly so the next block's PE
                # matmuls don't wait on the normalization chain
                o_sb = []
                for cc in range(CCH):
                    t = fin.tile([P, IB], F32, name=f"o_sb{cc}", tag=f"osb{cc}")
                    nc.vector.tensor_copy(t, o_ps[cc])
                    o_sb.append(t)

                # denominator -> [128, IB] broadcast (PE) + reciprocal (DVE)
                sums_row = fin.tile([1, IB], F32, name="sums_row", tag="sums_row")
                nc.vector.tensor_copy(sums_row, sums_ps[0:1, :])
                rb_ps = sm_psum.tile([P, IB], F32, name="rb_ps", tag="rb")
                nc.tensor.matmul(rb_ps, lhsT=ones_f[0:1, :], rhs=sums_row,
                                 start=True, stop=True)
                rb = fin.tile([P, IB], F32, name="rb", tag="rbs")
                nc.vector.reciprocal(rb, rb_ps)

                for oc in range(CCH):
                    t = fin.tile([P, IB], F32, name="t_sb", tag="t_sb")
                    nc.vector.tensor_mul(t, o_sb[oc], rb)
                    out_sb = fin.tile([P, IB], F32, name="out_sb", tag="out_sb")
                    nc.vector.tensor_add(out_sb, t, resid[:, oc, isl])
                    nc.sync.dma_start(
                        out=y_d[oc * P:(oc + 1) * P, isl], in_=out_sb
                    )
    nc.finalize()
    return nc


_NC_CACHE = {}


def _get_nc(mm_dtype="f32r"):
    if mm_dtype not in _NC_CACHE:
        _NC_CACHE[mm_dtype] = build_nc(mm_dtype)
    return _NC_CACHE[mm_dtype]


def make_in_maps(inputs):
    """Shard full inputs into per-core input maps (host-side weight folding)."""
    x = np.asarray(inputs["x"], np.float32).reshape(B, C, N)
    gamma = np.asarray(inputs["gamma"], np.float32)
    beta = np.asarray(inputs["beta"], np.float32)
    wq = np.asarray(inputs["wq"], np.float64)
    bq = np.asarray(inputs["bq"], np.float64)
    wk = np.asarray(inputs["wk"], np.float64)
    wv = np.asarray(inputs["wv"], np.float64)
    bv = np.asarray(inputs["bv"], np.float64)
    wo = np.asarray(inputs["wo"], np.float64)
    bo = np.asarray(inputs["bo"], np.float32)

    # S^T = xn^T (wq^T wk) xn + (wk^T bq) broadcast over keys
    wqk = np.ascontiguousarray((wq.T @ wk).astype(np.float32))      # [c, c']
    bqk = (wk.T @ bq).astype(np.float32)                            # [c']
    # out = (wo wv xn + wo bv) A_n^T
    wovT = np.ascontiguousarray((wo @ wv).T.astype(np.float32))     # [c', o]
    bvp = (wo @ bv).astype(np.float32)                              # [o]

    gind = np.zeros((CCH, P, GROUPS), np.float32)
    for ch in range(CCH):
        for p in range(P):
            gind[ch, p, (ch * P + p) // GSZ] = 1.0
    gindT = np.ascontiguousarray(gind.transpose(0, 2, 1))

    shared = {
        "wqk": wqk, "wovT": wovT,
        "gamma": gamma, "beta": beta,
        "bqk": bqk, "bvp": bvp, "bo": bo,
        "gind": gind, "gindT": gindT,
    }
    in_maps = []
    for core in range(NCORES):
        b, h = divmod(core, QSPLIT)
        if h == 0:
            xc = x[b]
        else:
            xc = np.concatenate(
                [x[b][:, h * NQ:(h + 1) * NQ], x[b][:, :h * NQ],
                 x[b][:, (h + 1) * NQ:]], axis=1,
            )
        in_maps.append({"x": np.ascontiguousarray(xc), **shared})
    return in_maps


def gather_output(results):
    y = np.empty((B, C, N), np.float32)
    for core in range(NCORES):
        b, h = divmod(core, QSPLIT)
        y[b][:, h * NQ:(h + 1) * NQ] = results[core]["y"]
    return y.reshape(B, C, H, W)


def _run_traced(nc, in_maps, core_ids, tmpdir=None):
    """Replicates run_bass_kernel_spmd's axon trace branch; this image
    lacks antenv.axon_hooks, so drive the NTFF hook via ctypes directly."""
    import glob
    import tempfile

    import gauge.profiler
    from concourse import bass2jax
    from concourse._compat import FishPath
    from concourse.bass_utils import BassKernelResults, _process_ntff_profile
    from trn_agent_boot.trn_boot import _ntff_profile_via_ctypes

    hook = _ntff_profile_via_ctypes("/opt/axon/libaxon_pjrt.so")
    if tmpdir is None:
        tmpdir = tempfile.mkdtemp(prefix="bassprof_")
    if hook is None:
        results = bass2jax.run_bass_via_pjrt(nc, in_maps, n_cores=len(core_ids))
        return BassKernelResults(results, None, None, None)
    with hook(tmpdir, [0]):
        results = bass2jax.run_bass_via_pjrt(nc, in_maps, n_cores=len(core_ids))
    if not glob.glob(f"{tmpdir}/*_body*.ntff"):
        print(f"no NTFF produced in {tmpdir}")
        return BassKernelResults(results, None, None, None)
    profile = gauge.profiler.Profile(
        profile_path=FishPath(tmpdir),
        kernel_dev_mode=True,
        profile_on_exit=False,
        bass_kernel=nc.m,
        offline_processing=True,
        fname="*_body*",
        metadata={},
    )
    return _process_ntff_profile(
        profile, tmpdir, nc, core_ids, None, False, {}, False
    ).as_bass_kernel_results(results)


def run_spmd(inputs, trace=False, mm_dtype="bf16+fp8", tmpdir=None):
    from concourse.bass_utils import run_bass_kernel_spmd

    nc = _get_nc(mm_dtype)
    in_maps = make_in_maps(inputs)
    if trace:
        res = _run_traced(nc, in_maps, list(range(NCORES)), tmpdir=tmpdir)
    else:
        res = run_bass_kernel_spmd(nc, in_maps, list(range(NCORES)), trace=False)
    return gather_output(res.results), res


def kernel(**inputs) -> np.ndarray:
    out, _ = run_spmd(inputs, trace=False, mm_dtype="bf16+fp8")
    return out



# revision 6
# speedup vs baseline: 1.2555x; 1.2555x over previous
"""Trainium2 Bass kernel: GroupNorm + single-head self-attention block.

Reference computation (per batch b):
    xn = GroupNorm(x, 16 groups, eps=1e-5) * gamma + beta
    q/k/v = W @ xn + b          (1x1 conv == channel matmul), [C, N]
    S = (q^T k) / sqrt(C)       [N, N]
    A = softmax_j(S)
    O = v @ A^T                 [C, N]
    y = wo @ O + bo + x

Shapes: B=4, C=256, H=W=64 -> N=4096.

Sharding: 8 cores = 4 batches x 2 query-halves.  Each core receives the
full x[b] with its query half permuted to the front, computes xn / v'
for all N keys and runs attention for its 2048 queries (SPMD).

Algebraic restructuring (host-side, exact):
  - S^T[j,i] = xn^T WQK xn + (wk^T bq)  with WQK = wq^T wk folded on the
    host (bk's contribution is softmax-invariant and dropped).  WQK and
    bqk are pre-scaled by QK_PRESCALE = 8/(16 ln2) so device scores live
    directly in fp8-e4m3 "bit" units (see exp trick below).
  - wo is folded into v: WOV = wo wv.  The bias (wo bv) is pulled out of
    the attention matmul entirely: since softmax rows sum to 1,
    (v'+b) A_n^T = v' A_n^T + b, so it lands in the residual bias.

Device numerics (all big matmuls fp8-e4m3 DoubleRow, K=256 per pass):
  - qk8 = WQK8^T xn8 + bqk'         [c', i]  (fp8, prescaled)
  - S'  = xn8^T qk8                 per key tile, PSUM f32
  - attention weights at = exp(S'/QK_PRESCALE/16 - ln16) as fp8:
      * ACT engine pairs: exact exp (scale=ln2/8, bias=-ln16) -> fp8
      * DVE engine pairs: Schraudolph bit trick -- for e4m3,
        bits(v) ~= 8 log2(v) + 56, so bits(exp(s~ - ln16)) ~= S' + 24.
        One tensor_scalar (add 24, max 0) with uint8 output, bitcast to
        fp8.  The PWL mean bias cancels in the softmax normalization.
    Splitting exp across both engines removes the ACT throughput wall.
  - denominator: DoubleRow ones-matmul with M=128 -> the PSUM result is
    already broadcast across all partitions; reciprocal_approx_fast.
  - out = vT8^T at (DoubleRow), normalized and fused with the residual
    via scalar_tensor_tensor: y = (x + (bo + wo bv)) + o * recip.
"""

import sys

sys.path.insert(0, "/opt/trn_rl_repo")

from contextlib import ExitStack

import numpy as np

import concourse.bacc as bacc
import concourse.bass as bass
import concourse.mybir as mybir
import concourse.tile as tile

B, C, H, W = 4, 256, 64, 64
N = H * W              # keys per batch
GROUPS = 16
EPS = 1e-5
NCORES = 8
QSPLIT = NCORES // B   # query shards per batch
NQ = N // QSPLIT       # queries per core
P = 128
CCH = C // P           # channel chunks (2)
IB = 512               # query block (one PSUM bank of f32)
NIB = NQ // IB         # query blocks per core
NJT = N // P           # key tiles (32)
NPAIR = NJT // 2       # key-tile pairs (16)
GSZ = C // GROUPS      # channels per group (16)
NS = N // 512          # bn_stats subgroups per chunk (8)
XBLK = N // 4          # xn8 column block (1024)

LN2 = 0.6931471805599453
QK_PRESCALE = 8.0 / (16.0 * LN2)   # folds 1/sqrt(C) and the e4m3 bit scale
ACT_SCALE = LN2 / 8.0              # exact-exp path: exp(S'*ACT_SCALE - ln16)
NEG_LN16 = -2.772588722239781
EXP_OFFSET = 24.0                  # 56 - 8*ln16/ln2

F32 = mybir.dt.float32
BF16 = mybir.dt.bfloat16
FP8 = mybir.dt.float8e4
U8 = mybir.dt.uint8
AF = mybir.ActivationFunctionType
OP = mybir.AluOpType
DR = mybir.MatmulPerfMode.DoubleRow


def build_nc(exp_mode: str = "mixed"):
    """Emit the single-core SPMD program."""
    nc = bacc.Bacc()

    x_d = nc.declare_dram_parameter("x", [C, N], F32, isOutput=False)
    wqk_d = nc.declare_dram_parameter("wqk8", [P, CCH, C], U8, isOutput=False)
    wov_d = nc.declare_dram_parameter("wov8", [P, CCH, C], U8, isOutput=False)
    gamma_d = nc.declare_dram_parameter("gamma", [C], F32, isOutput=False)
    beta_d = nc.declare_dram_parameter("beta", [C], F32, isOutput=False)
    bqkp_d = nc.declare_dram_parameter("bqkp", [C], F32, isOutput=False)
    bob_d = nc.declare_dram_parameter("bob", [C], F32, isOutput=False)
    gind_d = nc.declare_dram_parameter("gind", [CCH, P, GROUPS], F32, isOutput=False)
    gindT_d = nc.declare_dram_parameter("gindT", [CCH, GROUPS, P], F32, isOutput=False)
    y_d = nc.declare_dram_parameter("y", [C, NQ], F32, isOutput=True)

    with tile.TileContext(nc) as tc, ExitStack() as ctx:
        const = ctx.enter_context(tc.tile_pool(name="const", bufs=1))
        data = ctx.enter_context(tc.tile_pool(name="data", bufs=1))

        # ---- constants / weights ----
        ones_f = const.tile([P, P], F32, name="ones_f")
        nc.vector.memset(ones_f, 1.0)
        ones2b = const.tile([P, 2], BF16, name="ones2b")
        nc.vector.memset(ones2b, 1.0)
        warm_src = const.tile([P, 512], BF16, name="warm_src")
        nc.vector.memset(warm_src, 0.0)
        ones_dr_u = const.tile([P, 2, P], U8, name="ones_dr_u")
        nc.vector.memset(ones_dr_u, 56)   # fp8e4m3 bits of 1.0
        neg_ln16 = const.tile([P, 1], F32, name="neg_ln16")
        nc.vector.memset(neg_ln16, NEG_LN16)

        wqk8 = const.tile([P, CCH, C], U8, name="wqk8")
        nc.scalar.dma_start(out=wqk8, in_=wqk_d[:, :, :])
        wov8 = const.tile([P, CCH, C], U8, name="wov8")
        nc.scalar.dma_start(out=wov8, in_=wov_d[:, :, :])

        def load_vec(handle, nm):
            tiles = []
            for ch in range(CCH):
                t = const.tile([P, 1], F32, name=f"{nm}{ch}")
                nc.scalar.dma_start(
                    out=t, in_=handle[ch * P:(ch + 1) * P].unsqueeze(1)
                )
                tiles.append(t)
            return tiles

        gamma = load_vec(gamma_d, "gamma")
        beta = load_vec(beta_d, "beta")
        bqkp = load_vec(bqkp_d, "bqkp")
        bob = load_vec(bob_d, "bob")

        gind = []
        gindT = []
        for ch in range(CCH):
            gi = const.tile([P, GROUPS], F32, name=f"gind{ch}")
            nc.scalar.dma_start(out=gi, in_=gind_d[ch])
            gind.append(gi)
            gt = const.tile([GROUPS, P], F32, name=f"gindT{ch}")
            nc.scalar.dma_start(out=gt, in_=gindT_d[ch])
            gindT.append(gt)

        # ---- persistent data tiles ----
        xf = data.tile([P, CCH, N], F32, name="xf")
        xn8 = data.tile([P, CCH, N], FP8, name="xn8")
        qk8 = data.tile([P, CCH, NQ], FP8, name="qk8")
        vT8 = data.tile([P, NPAIR, 2, C], FP8, name="vT8")

        # Greedy ACT/DVE load balancer for exp + PSUM->fp8 conversions.
        est = {"act": 1.5, "dve": 15.0}  # emission-time backlog estimate (us)

        def pick():
            return "act" if est["act"] <= est["dve"] else "dve"

        with tc.tile_pool(name="warm_psum", bufs=1, space="PSUM") as warm_psum:
            warm_ps = warm_psum.tile([P, 512], F32, name="warm_ps")

            def warm_burst(n):
                for _ in range(n):
                    nc.tensor.matmul(
                        warm_ps[:2, :512], lhsT=ones2b, rhs=warm_src,
                        start=True, stop=True, skip_group_check=True,
                    )

            def ping(rhs):
                # data-dependent f32 matmul keeps the HAM activity window fed
                w = rhs.shape[-1]
                k = rhs.shape[0]
                nc.tensor.matmul(
                    warm_ps[:2, :w], lhsT=ones_f[:k, 0:2], rhs=rhs,
                    start=True, stop=True, skip_group_check=True,
                )

            # PE HAM: the clock gate opens after ~3.4us of sustained activity
            # and re-throttles after an idle window; burn a dense burst at
            # t=0 and drip data-dependent pings through the prologue.
            warm_burst(26)

            # ---- x DMA: 8 large transfers across 2 idle queues ----
            for k in range(8):
                blk, ch = divmod(k, 2)
                eng = nc.sync if k % 2 == 0 else nc.gpsimd
                eng.dma_start(
                    out=xf[:, ch, blk * XBLK:(blk + 1) * XBLK],
                    in_=x_d[ch * P:(ch + 1) * P, blk * XBLK:(blk + 1) * XBLK],
                )

            # ---- GroupNorm ----
            with tc.tile_pool(name="gn_psum", bufs=1, space="PSUM") as gn_psum, \
                 tc.tile_pool(name="gn_sb", bufs=1) as gn_sb:
                st6 = [
                    gn_sb.tile([P, NS, 6], F32, name=f"st6_{ch}")
                    for ch in range(CCH)
                ]
                for blk in range(4):
                    for ch in range(CCH):
                        for h in range(2):
                            sg = 2 * blk + h
                            nc.vector.bn_stats(
                                out=st6[ch][:, sg, :],
                                in_=xf[:, ch, sg * 512:(sg + 1) * 512],
                            )
                            ping(st6[ch][:, sg, :])
                pc = []
                for ch in range(CCH):
                    mv = gn_sb.tile([P, 2], F32, name=f"mv{ch}")
                    nc.vector.bn_aggr(out=mv, in_=st6[ch])
                    pcs = gn_sb.tile([P, 2], F32, name=f"pcs{ch}")
                    nc.vector.tensor_copy(pcs[:, 0:1], mv[:, 0:1])
                    msq = gn_sb.tile([P, 1], F32, name=f"msq{ch}")
                    nc.vector.tensor_mul(msq, mv[:, 0:1], mv[:, 0:1])
                    nc.vector.tensor_add(pcs[:, 1:2], mv[:, 1:2], msq)
                    pc.append(pcs)
                    ping(pcs)

                gs_ps = gn_psum.tile([GROUPS, 2], F32, name="gs_ps")
                for ch in range(CCH):
                    nc.tensor.matmul(
                        gs_ps, lhsT=gind[ch], rhs=pc[ch],
                        start=(ch == 0), stop=(ch == CCH - 1),
                    )
                gs = gn_sb.tile([GROUPS, 2], F32, name="gs")
                nc.scalar.mul(gs, gs_ps, 1.0 / GSZ)
                gvar = gn_sb.tile([GROUPS, 1], F32, name="gvar")
                gmsq = gn_sb.tile([GROUPS, 1], F32, name="gmsq")
                nc.vector.tensor_mul(gmsq, gs[:, 0:1], gs[:, 0:1])
                nc.vector.tensor_sub(gvar, gs[:, 1:2], gmsq)
                gstd = gn_sb.tile([GROUPS, 1], F32, name="gstd")
                eps_t = gn_sb.tile([GROUPS, 1], F32, name="eps_t")
                nc.vector.memset(eps_t, EPS)
                nc.scalar.activation(
                    out=gstd, in_=gvar, func=AF.Sqrt, bias=eps_t, scale=1.0
                )
                gmr = gn_sb.tile([GROUPS, 2], F32, name="gmr")
                nc.vector.tensor_copy(gmr[:, 0:1], gs[:, 0:1])
                nc.vector.reciprocal(gmr[:, 1:2], gstd)
                ping(gmr)

                scale = []
                shift = []
                for ch in range(CCH):
                    cb_ps = gn_psum.tile([P, 2], F32, name="cb_ps", tag="cb_ps")
                    nc.tensor.matmul(cb_ps, lhsT=gindT[ch], rhs=gmr,
                                     start=True, stop=True)
                    cb = gn_sb.tile([P, 2], F32, name=f"cb{ch}")
                    nc.vector.tensor_copy(cb, cb_ps)
                    sc = const.tile([P, 1], F32, name=f"scale{ch}")
                    nc.vector.tensor_mul(sc, gamma[ch], cb[:, 1:2])
                    sh = const.tile([P, 1], F32, name=f"shift{ch}")
                    nc.vector.tensor_mul(sh, cb[:, 0:1], sc)
                    nc.vector.tensor_sub(sh, beta[ch], sh)
                    scale.append(sc)
                    shift.append(sh)
                    ping(cb)

                # xn8 = x * scale + shift, quantized to fp8 (column blocks so
                # the projections can start on early columns)
                for blk in range(4):
                    for ch in range(CCH):
                        nc.vector.tensor_scalar(
                            out=xn8[:, ch, blk * XBLK:(blk + 1) * XBLK],
                            in0=xf[:, ch, blk * XBLK:(blk + 1) * XBLK],
                            scalar1=scale[ch], scalar2=shift[ch],
                            op0=OP.mult, op1=OP.add,
                        )
                    ping(scale[ch])

        # ---- projections + attention ----
        wqk8f = wqk8.bitcast(FP8)
        wov8f = wov8.bitcast(FP8)
        ones_dr = ones_dr_u.bitcast(FP8)

        with tc.tile_pool(name="pj_psum", bufs=1, space="PSUM") as pj_psum, \
             tc.tile_pool(name="st_psum", bufs=2, space="PSUM") as st_psum, \
             tc.tile_pool(name="o_psum", bufs=1, space="PSUM") as o_psum, \
             tc.tile_pool(name="sm_psum", bufs=1, space="PSUM") as sm_psum, \
             tc.tile_pool(name="at_pool", bufs=6) as at_pool, \
             tc.tile_pool(name="fin", bufs=2) as fin:

            def qk_proj(blk, oc):
                isl = slice(blk * IB, (blk + 1) * IB)
                ps = pj_psum.tile([P, IB], F32, name="qk_ps", tag="pj")
                nc.tensor.matmul(
                    ps, lhsT=wqk8f[:, :, oc * P:(oc + 1) * P],
                    rhs=xn8[:, :, isl], start=True, stop=True, perf_mode=DR,
                )
                eng = pick()
                if eng == "act":
                    nc.scalar.activation(
                        out=qk8[:, oc, isl], in_=ps, func=AF.Identity,
                        bias=bqkp[oc], scale=1.0,
                    )
                    est["act"] += 0.72
                else:
                    nc.vector.tensor_scalar_add(
                        qk8[:, oc, isl], ps, scalar1=bqkp[oc]
                    )
                    est["dve"] += 0.66

            def vt_proj(pg):
                ps = pj_psum.tile([P, 2, C], F32, name="vt_ps", tag="pj")
                for m in range(2):
                    jt = 2 * pg + m
                    nc.tensor.matmul(
                        ps[:, m, :],
                        lhsT=xn8[:, :, jt * P:(jt + 1) * P],
                        rhs=wov8f, start=True, stop=True, perf_mode=DR,
                    )
                eng = pick()
                if eng == "act":
                    nc.scalar.activation(
                        out=vT8[:, pg].rearrange("p a b -> p (a b)"),
                        in_=ps.rearrange("p a b -> p (a b)"),
                        func=AF.Copy, scale=1.0,
                    )
                    est["act"] += 0.72
                else:
                    nc.vector.tensor_copy(vT8[:, pg], ps)
                    est["dve"] += 0.66

            # prologue projections: queries block 0 + first two key pairs
            qk_proj(0, 0)
            qk_proj(0, 1)
            vt_proj(0)
            vt_proj(1)

            # proj tasks interleaved into attention block 0 (and qk blocks
            # for ib+1 interleaved into block ib)
            def interleave(ib, p):
                if ib == 0:
                    if p <= 13:
                        vt_proj(p + 2)
                    elif p == 14:
                        qk_proj(1, 0)
                    elif p == 15:
                        qk_proj(1, 1)
                elif ib < NIB - 1:
                    if p == 0:
                        qk_proj(ib + 1, 0)
                    elif p == 8:
                        qk_proj(ib + 1, 1)

            for ib in range(NIB):
                isl = slice(ib * IB, (ib + 1) * IB)
                sums_ps = sm_psum.tile([P, IB], F32, name="sums_ps", tag="sums")
                o_ps = [
                    o_psum.tile([P, IB], F32, name=f"o_ps{cc}", tag=f"o{cc}")
                    for cc in range(CCH)
                ]
                ats = {}
                for p in range(NPAIR + 1):
                    if p < NPAIR:
                        stp = st_psum.tile([P, 2, IB], F32, name="stp", tag="st")
                        for m in range(2):
                            jt = 2 * p + m
                            nc.tensor.matmul(
                                stp[:, m, :],
                                lhsT=xn8[:, :, jt * P:(jt + 1) * P],
                                rhs=qk8[:, :, isl],
                                start=True, stop=True, perf_mode=DR,
                            )
                        interleave(ib, p)
                        atp = at_pool.tile([P, 2, IB], FP8, name="atp", tag="at")
                        eng = "act" if exp_mode == "act" else pick()
                        if eng == "act":
                            nc.scalar.activation(
                                out=atp.rearrange("p a b -> p (a b)"),
                                in_=stp.rearrange("p a b -> p (a b)"),
                                func=AF.Exp, scale=ACT_SCALE, bias=neg_ln16,
                            )
                            est["act"] += 1.15
                        else:
                            # e4m3 bit-trick exp2: bits = max(S' + 24, 0)
                            nc.vector.tensor_scalar(
                                out=atp.bitcast(U8).rearrange("p a b -> p (a b)"),
                                in0=stp.rearrange("p a b -> p (a b)"),
                                scalar1=EXP_OFFSET, scalar2=0.0,
                                op0=OP.add, op1=OP.max,
                            )
                            est["dve"] += 1.19
                        ats[p] = atp
                    if p >= 1:
                        pg = p - 1
                        atp = ats.pop(pg)
                        nc.tensor.matmul(
                            sums_ps, lhsT=ones_dr, rhs=atp,
                            start=(pg == 0), stop=(pg == NPAIR - 1),
                            perf_mode=DR,
                        )
                        for cc in range(CCH):
                            nc.tensor.matmul(
                                o_ps[cc],
                                lhsT=vT8[:, pg, :, cc * P:(cc + 1) * P],
                                rhs=atp,
                                start=(pg == 0), stop=(pg == NPAIR - 1),
                                perf_mode=DR,
                            )

                # normalization + residual epilogue (sums_ps rows are already
                # the broadcast denominator thanks to the M=128 ones matmul)
                rb = fin.tile([P, IB], F32, name="rb", tag="rb")
                nc.vector.reciprocal_approx_fast(out=rb, in_=sums_ps)
                est["dve"] += 0.7
                for cc in range(CCH):
                    t = fin.tile([P, IB], F32, name="t_sb", tag="t_sb")
                    nc.vector.tensor_mul(t, o_ps[cc], rb)
                    out_sb = fin.tile([P, IB], F32, name="out_sb", tag="out_sb")
                    nc.vector.scalar_tensor_tensor(
                        out=out_sb, in0=xf[:, cc, isl], scalar=bob[cc],
                        in1=t, op0=OP.add, op1=OP.add,
                    )
                    est["dve"] += 1.4
                    eng = nc.sync if cc == 0 else nc.gpsimd
                    eng.dma_start(out=y_d[cc * P:(cc + 1) * P, isl], in_=out_sb)
    nc.finalize()
    return nc


_NC_CACHE = {}


def _get_nc(exp_mode="mixed"):
    if exp_mode not in _NC_CACHE:
        _NC_CACHE[exp_mode] = build_nc(exp_mode)
    return _NC_CACHE[exp_mode]


def make_in_maps(inputs):
    """Shard full inputs into per-core input maps (host-side weight folding)."""
    import ml_dtypes

    def f8u(a):
        return np.ascontiguousarray(
            a.astype(np.float32).astype(ml_dtypes.float8_e4m3).view(np.uint8)
        )

    x = np.asarray(inputs["x"], np.float32).reshape(B, C, N)
    gamma = np.asarray(inputs["gamma"], np.float32)
    beta = np.asarray(inputs["beta"], np.float32)
    wq = np.asarray(inputs["wq"], np.float64)
    bq = np.asarray(inputs["bq"], np.float64)
    wk = np.asarray(inputs["wk"], np.float64)
    wv = np.asarray(inputs["wv"], np.float64)
    bv = np.asarray(inputs["bv"], np.float64)
    wo = np.asarray(inputs["wo"], np.float64)
    bo = np.asarray(inputs["bo"], np.float32)

    wqk = (wq.T @ wk) * QK_PRESCALE                  # [c, c'] prescaled
    bqkp = ((wk.T @ bq) * QK_PRESCALE).astype(np.float32)
    wov = wo @ wv                                    # [o, c']
    bob = (bo.astype(np.float64) + wo @ bv).astype(np.float32)

    # DoubleRow pair-chunk layouts
    wqk8 = f8u(wqk.reshape(CCH, P, C).transpose(1, 0, 2))        # [p, ch, c']
    wov8 = f8u(wov.T.reshape(CCH, P, C).transpose(1, 0, 2))      # [p, ch, o]

    gind = np.zeros((CCH, P, GROUPS), np.float32)
    for ch in range(CCH):
        for p in range(P):
            gind[ch, p, (ch * P + p) // GSZ] = 1.0
    gindT = np.ascontiguousarray(gind.transpose(0, 2, 1))

    shared = {
        "wqk8": wqk8, "wov8": wov8,
        "gamma": gamma, "beta": beta,
        "bqkp": bqkp, "bob": bob,
        "gind": gind, "gindT": gindT,
    }
    in_maps = []
    for core in range(NCORES):
        b, h = divmod(core, QSPLIT)
        if h == 0:
            xc = x[b]
        else:
            xc = np.concatenate(
                [x[b][:, h * NQ:(h + 1) * NQ], x[b][:, :h * NQ],
                 x[b][:, (h + 1) * NQ:]], axis=1,
            )
        in_maps.append({"x": np.ascontiguousarray(xc), **shared})
    return in_maps


def gather_output(results):
    y = np.empty((B, C, N), np.float32)
    for core in range(NCORES):
        b, h = divmod(core, QSPLIT)
        y[b][:, h * NQ:(h + 1) * NQ] = results[core]["y"]
    return y.reshape(B, C, H, W)


def _run_traced(nc, in_maps, core_ids, tmpdir=None):
    """Replicates run_bass_kernel_spmd's axon trace branch; this image
    lacks antenv.axon_hooks, so drive the NTFF hook via ctypes directly."""
    import glob
    import tempfile

    import gauge.profiler
    from concourse import bass2jax
    from concourse._compat import FishPath
    from concourse.bass_utils import BassKernelResults, _process_ntff_profile
    from trn_agent_boot.trn_boot import _ntff_profile_via_ctypes

    hook = _ntff_profile_via_ctypes("/opt/axon/libaxon_pjrt.so")
    if tmpdir is None:
        tmpdir = tempfile.mkdtemp(prefix="bassprof_")
    if hook is None:
        results = bass2jax.run_bass_via_pjrt(nc, in_maps, n_cores=len(core_ids))
        return BassKernelResults(results, None, None, None)
    with hook(tmpdir, [0]):
        results = bass2jax.run_bass_via_pjrt(nc, in_maps, n_cores=len(core_ids))
    if not glob.glob(f"{tmpdir}/*_body*.ntff"):
        print(f"no NTFF produced in {tmpdir}")
        return BassKernelResults(results, None, None, None)
    profile = gauge.profiler.Profile(
        profile_path=FishPath(tmpdir),
        kernel_dev_mode=True,
        profile_on_exit=False,
        bass_kernel=nc.m,
        offline_processing=True,
        fname="*_body*",
        metadata={},
    )
    return _process_ntff_profile(
        profile, tmpdir, nc, core_ids, None, False, {}, False
    ).as_bass_kernel_results(results)


def run_spmd(inputs, trace=False, mm_dtype="mixed", tmpdir=None):
    from concourse.bass_utils import run_bass_kernel_spmd

    nc = _get_nc(mm_dtype)
    in_maps = make_in_maps(inputs)
    if trace:
        res = _run_traced(nc, in_maps, list(range(NCORES)), tmpdir=tmpdir)
    else:
        res = run_bass_kernel_spmd(nc, in_maps, list(range(NCORES)), trace=False)
    return gather_output(res.results), res


def kernel(**inputs) -> np.ndarray:
    out, _ = run_spmd(inputs, trace=False, mm_dtype="mixed")
    return out
